# revision 29
# baseline (speedup 1.0000x reference)
"""Trainium2 Bass kernel for nn_CombinedLoss (LCCNet CombinedLoss).

Strategy
--------
The only heavy part is the point-cloud term: for each sample b,
    err_n = || (RT_inv_b - I) @ p_n ||   over N=200000 homogeneous points,
    loss_pc = sum_b mean_n err_n.
Everything else (pose loss, 4x4 transform algebra) is O(B) scalar work done
on the host in float64.

Per sample the displacement is d = A3 @ q + a4 with q = (x,y,z). Using a
column-pivoted QR A3 = Q R (orthogonal Q preserves the norm):
    err^2 = (s1*(u1 + a*u2 + b*u3) + b1)^2
          + (s2*(u2 + g*u3) + b2)^2
          + b3^2                      (A3 is rank 2 -> R[2,2] = 0)
where u = permuted coords, s_i = R[i,i], (b1,b2,b3) = Q^T a4, and the ratios
a,b,g are bounded by 1 thanks to pivoting (fp16-safe).

Device mapping (8 cores, data-parallel over batch; per core 4 samples packed
as 128 partitions = 4 samples x 32 partition-rows, 6250 points per row):
  - The per-sample scalars live in a [128, 1] column each (constant within a
    sample's 32 partitions), so ONE instruction covers all 4 samples.
  - DMA (gpsimd SWDGE, fp32->fp16 cast in flight) streams the free dim in
    chunks; compute starts when chunk 0 lands.
  - DVE per chunk: 3 tensor_scalar multiplies (4x perf mode) + 3
    tensor_tensor adds (2x mode) for the combines, + 1 add for s12.
    Instruction order keeps >=1 op between same-engine RAW pairs, so no
    drain() is needed.
  - ACT per chunk: 2 Square activations (fused per-partition scale+bias) +
    1 Sqrt with per-partition bias b3^2 and free-dim accum_out.
  - SP: waits for all Sqrts, DMAs the [128, n_chunks] accumulator out.
  - Host: final sums in float64, pose loss, combine.
"""

import numpy as np

B = 32
N = 200000
NCORES = 8
SPC = B // NCORES          # samples per core
NPART = 128
ROWS = 32                  # partition-rows per sample
PPTS = N // ROWS           # points per partition-row = 6250
# free-dim compute chunks (sum = PPTS, all even). One dma_start each (384
# descriptors spread over all 16 SDMA queues). Shape rationale, from
# measured DMA behavior: a tiny chunk 0 gets compute started ~2us earlier
# (first-DMA landing has a ~4.5us fixed latency + ~3.3ns/point); the big
# middle chunks give 6.2KB descriptors (per-queue throughput degrades
# sharply below ~4KB: ~90ns fixed per descriptor); the small final chunks
# keep the post-DMA compute tail short.
CHUNKS = [192, 1564, 1564, 1564, 768, 450, 148]
NCHUNK = len(CHUNKS)
OFFS = [0]
for _f in CHUNKS:
    OFFS.append(OFFS[-1] + _f)
# sqrt grouping: (first_chunk, last_chunk) half-open chunk ranges, one
# accumulator column each; arranged so only a narrow sqrt lands in the tail
SQRT_GROUPS = [(0, 3), (3, 5), (5, 7)]
NCONST = 8                 # per-sample scalar constants

_CACHED_NC = None


def _quat_to_rot(q):
    """Normalized quaternion [w,x,y,z] -> 3x3 rotation matrix (float64)."""
    q = q / np.linalg.norm(q)
    w, x, y, z = q
    return np.array([
        [1 - 2*y*y - 2*z*z, 2*x*y - 2*z*w,     2*x*z + 2*y*w],
        [2*x*y + 2*z*w,     1 - 2*x*x - 2*z*z, 2*y*z - 2*x*w],
        [2*x*z - 2*y*w,     2*y*z + 2*x*w,     1 - 2*x*x - 2*y*y],
    ])


def _pivoted_qr(A3):
    """Column-pivoted QR of a 3x3 matrix (float64). A3[:, piv] = Q @ R.

    Modified Gram-Schmidt with greedy max-residual-norm pivoting, which
    guarantees |R[i, j]| <= |R[i, i]| for j > i (bounded ratios)."""
    cols = {c: A3[:, c].astype(np.float64).copy() for c in range(3)}
    coeff = {c: np.zeros(3) for c in range(3)}   # coeff[c][i] = Q[:,i].A3[:,c]
    remaining = [0, 1, 2]
    piv = []
    Q = np.zeros((3, 3))
    for i in range(3):
        cbest = max(remaining, key=lambda c: float(np.dot(cols[c], cols[c])))
        remaining.remove(cbest)
        piv.append(cbest)
        v = cols[cbest]
        nrm = np.sqrt(np.dot(v, v))
        if nrm < 1e-300:
            # Degenerate column: pick any unit vector orthogonal to prior qs.
            for basis in np.eye(3):
                w = basis - Q[:, :i] @ (Q[:, :i].T @ basis)
                if np.dot(w, w) > 1e-12:
                    v = w
                    break
            nrm = np.sqrt(np.dot(v, v))
        q = v / nrm
        Q[:, i] = q
        for c in [cbest] + remaining:
            proj = float(np.dot(q, cols[c]))
            coeff[c][i] = proj
            cols[c] = cols[c] - proj * q
    R = np.stack([coeff[c] for c in piv], axis=1)
    return Q, R, piv


def _per_sample_host(tt, tr, te, re_):
    """Returns (piv, consts[8] float32) for one sample."""
    R_t = _quat_to_rot(tr.astype(np.float64))
    R_p = _quat_to_rot(re_.astype(np.float64))
    A3 = R_p.T @ R_t - np.eye(3)
    a4 = R_p.T @ (tt.astype(np.float64) - te.astype(np.float64))
    Q, R, piv = _pivoted_qr(A3)
    b4 = Q.T @ a4
    r11, r12, r13 = R[0, 0], R[0, 1], R[0, 2]
    r22, r23 = R[1, 1], R[1, 2]
    alpha = r12 / r11 if abs(r11) > 1e-30 else 0.0
    beta = r13 / r11 if abs(r11) > 1e-30 else 0.0
    gamma = r23 / r22 if abs(r22) > 1e-30 else 0.0
    consts = np.array([alpha, beta, gamma, r11, b4[0], r22, b4[1],
                       b4[2] ** 2])
    return piv, consts


def _build_nc():
    """Raw-Bass kernel (no TileContext): all synchronization is standalone
    wait_ge instructions on explicit semaphores.

    Engine programs:
      Pool (SWDGE): consts DMA, then one point-cloud DMA per free-dim chunk
          (fp32->fp16 cast in flight), each signalling its own semaphore.
          SWDGE emits descriptors in FIFO order -> chunks land in order.
      DVE: per chunk, 3 tensor_scalar multiplies + 3 tensor_tensor adds
          (combines; f1 signals), plus the s12 = t1 + t2 add (signals).
          s12 of chunk k is emitted after the combines of chunk k+1 so DVE
          never stalls on ACT, and every same-engine RAW pair has >=1
          instruction in between (no drain needed).
      ACT: per chunk, 2 Square activations (fused per-partition scale+bias;
          t2 signals) and 1 Sqrt with bias b3^2 and accum_out -> acc column
          (signals).
      SP: waits for all Sqrt results, DMAs acc out, waits for completion.
    """
    import concourse.bass as bass
    from concourse import mybir
    from contextlib import ExitStack

    f16, f32 = mybir.dt.float16, mybir.dt.float32
    Alu = mybir.AluOpType
    Act = mybir.ActivationFunctionType

    nc = bass.Bass("TRN2", target_bir_lowering=False, debug=False,
                   num_devices=NCORES)
    # piece-major flat layout: piece j of chunk k stores 3 contiguous
    # [128, Pf] blocks (one per coord), so every dma_start reads one
    # sequential HBM range.
    pc = nc.dram_tensor("pc", [NPART * 3 * PPTS], f32,
                        kind="ExternalInput").ap()
    # per-partition scalars [128, 8]: (alpha, beta, gamma, s1, b1, s2, b2,
    # b3sq), fp32; constant within each sample's 32 partitions.
    consts = nc.dram_tensor("consts", [NPART, NCONST], f32,
                            kind="ExternalInput").ap()
    ngroups = len(SQRT_GROUPS)
    acc_out0 = nc.dram_tensor("acc0", [NPART, ngroups - 1], f32,
                              kind="ExternalOutput").ap()
    acc_out1 = nc.dram_tensor("acc1", [NPART, 1], f32,
                              kind="ExternalOutput").ap()

    with ExitStack() as ctx:
        E = ctx.enter_context
        ct = E(nc.sbuf_tensor("ct", [NPART, NCONST], f32))
        acc = E(nc.sbuf_tensor("acc_sb", [NPART, ngroups], f32))
        scr = E(nc.sbuf_tensor("scr", [NPART, 4], f16))

        def tiles(nm, mult=1):
            return [E(nc.sbuf_tensor(f"{nm}{k}", [NPART, mult * CHUNKS[k]],
                                     f16)) for k in range(NCHUNK)]

        us = tiles("u", 3)
        a1s = tiles("a1_")
        aa = tiles("aa_", 2)       # [a2 | a3]
        cc12 = tiles("c12_", 2)    # [c1 | c2]
        f1s = tiles("f1_")
        t1s = tiles("t1_")
        t2s = tiles("t2_")
        # s12 results for all chunks live in ONE tile so sqrt can span
        # several chunks in a single activation
        s12all = E(nc.sbuf_tensor("s12all", [NPART, PPTS], f16))
        esall = E(nc.sbuf_tensor("esall", [NPART, PPTS], f16))

        sem_u = [E(nc.semaphore(f"sem_u{k}")) for k in range(NCHUNK)]
        sem_dve = E(nc.semaphore("sem_dve"))
        sem_act = E(nc.semaphore("sem_act"))
        sem_out = E(nc.semaphore("sem_out"))
        block = E(nc.Block())

        def cst(i):
            return ct[:, i:i + 1]

        def s12sl(k):
            return s12all[:, OFFS[k]:OFFS[k + 1]]

        # --- engine emit orders ---
        # DVE: chunk k >= 1 hosts s12_{k-1} between its c12 and f1 ops
        # (also fixes the same-engine RAW distance); the last chunk's own
        # s12 trails at the end.
        inter = {k: k - 1 for k in range(1, NCHUNK)}
        post = {NCHUNK - 1: [NCHUNK - 1]}
        # ACT: squares per chunk; sqrt group g after squares of chunk
        # grp_after[g]; wide groups run while late DMAs are in flight
        grp_after = {}
        for gi, (a, bb) in enumerate(SQRT_GROUPS):
            grp_after[gi] = min(bb + 1, NCHUNK - 2) if bb < NCHUNK else NCHUNK
        act_order = []
        for k in range(NCHUNK):
            act_order.append(("sq", k))
            for gi in range(ngroups):
                if grp_after[gi] == k:
                    act_order.append(("grp", gi))
        emitted = {x[1] for x in act_order if x[0] == "grp"}
        for gi in range(ngroups):
            if gi not in emitted:
                act_order.append(("grp", gi))

        # --- semaphore tick bookkeeping (program order per engine) ---
        dve_c2, dve_f1, dve_s12 = {}, {}, {}
        act_t1, act_grp = {}, {}
        dve_n = act_n = 0
        for k in range(NCHUNK):
            dve_n += 1; dve_c2[k] = dve_n
            if k in inter:
                dve_n += 1; dve_s12[inter[k]] = dve_n
            dve_n += 1; dve_f1[k] = dve_n
            for j in post.get(k, []):
                dve_n += 1; dve_s12[j] = dve_n
        for op, idx in act_order:
            act_n += 1
            if op == "sq":
                act_t1[idx] = act_n
            else:
                act_grp[idx] = act_n

        @block.gpsimd
        def _(g):
            g.dma_start(ct[:], consts).then_inc(sem_u[0], 16)
            for k in range(NCHUNK):
                base = NPART * 3 * OFFS[k]
                g.dma_start(
                    us[k][:].rearrange("p (c f) -> p c f", c=3),
                    pc[base:base + 3 * NPART * CHUNKS[k]].rearrange(
                        "(c p f) -> p c f", c=3, p=NPART),
                ).then_inc(sem_u[k], 16)

        @block.vector
        def _(v):
            def s12(j):
                v.wait_ge(sem_act, act_t1[j])
                v.tensor_tensor(s12sl(j), t1s[j][:], t2s[j][:],
                                Alu.add).then_inc(sem_dve, 1)

            for k in range(NCHUNK):
                F = CHUNKS[k]
                u = us[k]
                u1, u2, u3 = (u[:, i * F:(i + 1) * F] for i in range(3))
                a2 = aa[k][:, 0:F]
                a3 = aa[k][:, F:2 * F]
                c1 = cc12[k][:, 0:F]
                c2 = cc12[k][:, F:2 * F]
                v.wait_ge(sem_u[k], 32 if k == 0 else 16)
                if k == 0:
                    # split form; every same-engine RAW pair >=1 apart
                    v.tensor_scalar(a3, u3, cst(2), None, Alu.mult)
                    v.tensor_scalar(a2, u3, cst(1), None, Alu.mult)
                    v.tensor_scalar(a1s[k][:], u2, cst(0), None, Alu.mult)
                    v.tensor_tensor(c1, u1, a2, Alu.add)
                    v.tensor_tensor(c2, u2, a3, Alu.add).then_inc(sem_dve, 1)
                    v.tensor_tensor(f1s[k][:], c1, a1s[k][:],
                                    Alu.add).then_inc(sem_dve, 1)
                else:
                    # merged form: one add produces [c1|c2]; the
                    # interleaved s12 spaces the c12->f1 RAW pair
                    v.tensor_scalar(a2, u3, cst(1), None, Alu.mult)
                    v.tensor_scalar(a3, u3, cst(2), None, Alu.mult)
                    v.tensor_scalar(a1s[k][:], u2, cst(0), None, Alu.mult)
                    v.tensor_tensor(cc12[k][:], u[:, 0:2 * F], aa[k][:],
                                    Alu.add).then_inc(sem_dve, 1)
                    s12(inter[k])
                    v.tensor_tensor(f1s[k][:], c1, a1s[k][:],
                                    Alu.add).then_inc(sem_dve, 1)
                for j in post.get(k, []):
                    s12(j)

        @block.scalar
        def _(s):
            # dummy activations preload the Square/Sqrt tables while the
            # first DMA is still in flight
            s.activation(scr[:, 2:4], scr[:, 0:2], Act.Square)
            s.activation(scr[:, 0:2], scr[:, 0:2], Act.Sqrt)

            def squares(k):
                F = CHUNKS[k]
                s.wait_ge(sem_dve, dve_c2[k])
                s.activation(t2s[k][:], cc12[k][:, F:2 * F], Act.Square,
                             bias=cst(6), scale=cst(5))
                s.wait_ge(sem_dve, dve_f1[k])
                s.activation(t1s[k][:], f1s[k][:], Act.Square,
                             bias=cst(4), scale=cst(3)).then_inc(sem_act, 1)

            def sqrt_grp(gi):
                a, bb = SQRT_GROUPS[gi]
                s.wait_ge(sem_dve, max(dve_s12[k] for k in range(a, bb)))
                s.activation(esall[:, OFFS[a]:OFFS[bb]],
                             s12all[:, OFFS[a]:OFFS[bb]], Act.Sqrt,
                             bias=cst(7),
                             accum_out=acc[:, gi:gi + 1]).then_inc(sem_act, 1)

            for op, idx in act_order:
                if op == "sq":
                    squares(idx)
                else:
                    sqrt_grp(idx)

        @block.sync
        def _(sp):
            # first accumulator columns go out as soon as their sqrt
            # groups are done; the last column right at the end
            half = ngroups - 1
            sp.wait_ge(sem_act, act_grp[half - 1])
            sp.dma_start(acc_out0, acc[:, 0:half]).then_inc(sem_out, 16)
            sp.wait_ge(sem_act, act_grp[ngroups - 1])
            sp.dma_start(acc_out1, acc[:, half:]).then_inc(sem_out, 16)
            sp.wait_ge(sem_out, 32)

    return nc


def _get_nc():
    global _CACHED_NC
    if _CACHED_NC is None:
        _CACHED_NC = _build_nc()
    return _CACHED_NC


def _kernel_impl(point_clouds, target_transl, target_rot, transl_err, rot_err,
                 trace=False):
    from concourse.bass_utils import run_bass_kernel_spmd

    pc = np.asarray(point_clouds)
    tt = np.asarray(target_transl, np.float64)
    tr = np.asarray(target_rot, np.float64)
    te = np.asarray(transl_err, np.float64)
    re_ = np.asarray(rot_err, np.float64)

    # ---- pose loss (host, float64, exact reference formulas) ----
    d = np.abs(te - tt)
    loss_transl = np.where(d < 1.0, 0.5 * d * d, d - 0.5).sum(axis=1).mean()

    rinv = tr * np.array([1.0, -1.0, -1.0, -1.0])
    q = re_
    w = q[:, 0]*rinv[:, 0] - q[:, 1]*rinv[:, 1] - q[:, 2]*rinv[:, 2] - q[:, 3]*rinv[:, 3]
    x = q[:, 0]*rinv[:, 1] + q[:, 1]*rinv[:, 0] + q[:, 2]*rinv[:, 3] - q[:, 3]*rinv[:, 2]
    y = q[:, 0]*rinv[:, 2] - q[:, 1]*rinv[:, 3] + q[:, 2]*rinv[:, 0] + q[:, 3]*rinv[:, 1]
    z = q[:, 0]*rinv[:, 3] + q[:, 1]*rinv[:, 2] - q[:, 2]*rinv[:, 1] + q[:, 3]*rinv[:, 0]
    angle = 2.0 * np.arctan2(np.sqrt(x*x + y*y + z*z), np.abs(w))
    loss_rot = (180.0 * angle / np.pi).mean()
    pose_loss = loss_transl + loss_rot

    # ---- per-sample transform constants (host) ----
    all_consts = np.zeros((B, NCONST), np.float32)
    all_piv = []
    for b in range(B):
        piv, consts = _per_sample_host(tt[b], tr[b], te[b], re_[b])
        all_consts[b] = consts
        all_piv.append(piv)

    # ---- build per-core inputs (permute coord rows per pivoting, pack
    #      4 samples x 32 partition-rows x 6250 points, chunk-major) ----
    pcp = np.stack([pc[b, all_piv[b], :] for b in range(B)])   # [B,3,N]
    pcp = pcp.reshape(NCORES, SPC, 3, ROWS, PPTS).transpose(0, 1, 3, 2, 4)
    pcp = pcp.reshape(NCORES, NPART, 3, PPTS)
    # chunk-major flat: [concat over chunks of [3, 128, F]]
    pcf = np.concatenate(
        [pcp[:, :, :, OFFS[k]:OFFS[k + 1]].transpose(0, 2, 1, 3)
         .reshape(NCORES, -1) for k in range(NCHUNK)], axis=1)
    in_maps = []
    for k in range(NCORES):
        cc = all_consts[k * SPC:(k + 1) * SPC]                 # [SPC, 8]
        in_maps.append({
            "pc": np.ascontiguousarray(pcf[k]),
            "consts": np.repeat(cc, ROWS, axis=0),             # [128, 8]
        })

    nc = _get_nc()
    res = run_bass_kernel_spmd(nc, in_maps, core_ids=list(range(NCORES)),
                               trace=trace)

    # ---- combine (host, float64) ----
    pcl_sum = 0.0
    for k in range(NCORES):
        pcl_sum += (res.results[k]["acc0"].astype(np.float64).sum()
                    + res.results[k]["acc1"].astype(np.float64).sum()) / N

    total = 0.5 * pose_loss + 0.5 * (pcl_sum / B)
    out = (np.float32(total), np.float32(loss_transl), np.float32(loss_rot),
           np.float32(pcl_sum / B))
    return out, res


def kernel(point_clouds, target_transl, target_rot, transl_err, rot_err):
    out, _ = _kernel_impl(point_clouds, target_transl, target_rot,
                          transl_err, rot_err)
    return out


# revision 31
# speedup vs baseline: 1.0256x; 1.0256x over previous
"""Trainium2 Bass kernel for nn_CombinedLoss (LCCNet CombinedLoss).

Strategy
--------
The only heavy part is the point-cloud term: for each sample b,
    err_n = || (RT_inv_b - I) @ p_n ||   over N=200000 homogeneous points,
    loss_pc = sum_b mean_n err_n.
Everything else (pose loss, 4x4 transform algebra) is O(B) scalar work done
on the host in float64.

Per sample the displacement is d = A3 @ q + a4 with q = (x,y,z). Using a
column-pivoted QR A3 = Q R (orthogonal Q preserves the norm):
    err^2 = (s1*(u1 + a*u2 + b*u3) + b1)^2
          + (s2*(u2 + g*u3) + b2)^2
          + b3^2                      (A3 is rank 2 -> R[2,2] = 0)
where u = permuted coords, s_i = R[i,i], (b1,b2,b3) = Q^T a4, and the ratios
a,b,g are bounded by 1 thanks to pivoting (fp16-safe).

Device mapping (8 cores, data-parallel over batch; per core 4 samples packed
as 128 partitions = 4 samples x 32 partition-rows, 6250 points per row):
  - The per-sample scalars live in a [128, 1] column each (constant within a
    sample's 32 partitions), so ONE instruction covers all 4 samples.
  - DMA (gpsimd SWDGE, fp32->fp16 cast in flight) streams the free dim in
    chunks; compute starts when chunk 0 lands.
  - DVE per chunk: 3 tensor_scalar multiplies (4x perf mode) + 3
    tensor_tensor adds (2x mode) for the combines, + 1 add for s12.
    Instruction order keeps >=1 op between same-engine RAW pairs, so no
    drain() is needed.
  - ACT per chunk: 2 Square activations (fused per-partition scale+bias) +
    1 Sqrt with per-partition bias b3^2 and free-dim accum_out.
  - SP: waits for all Sqrts, DMAs the [128, n_chunks] accumulator out.
  - Host: final sums in float64, pose loss, combine.
"""

import numpy as np

B = 32
N = 200000
NCORES = 8
SPC = B // NCORES          # samples per core
NPART = 128
ROWS = 32                  # partition-rows per sample
PPTS = N // ROWS           # points per partition-row = 6250
# free-dim compute chunks (sum = PPTS, all even). One dma_start each (384
# descriptors spread over all 16 SDMA queues). Shape rationale, from
# measured DMA behavior: a tiny chunk 0 gets compute started ~2us earlier
# (first-DMA landing has a ~4.5us fixed latency + ~3.3ns/point); the big
# middle chunks give 6.2KB descriptors (per-queue throughput degrades
# sharply below ~4KB: ~90ns fixed per descriptor); the small final chunks
# keep the post-DMA compute tail short.
CHUNKS = [256, 512, 896, 1280, 1564, 1214, 380, 148]
NCHUNK = len(CHUNKS)
OFFS = [0]
for _f in CHUNKS:
    OFFS.append(OFFS[-1] + _f)
# chunks using the 7-op split form (separate c1/c2 adds); the others use
# the 6-op merged form with an s12 interleaved between c12 and f1
SPLIT = {0, NCHUNK - 1}
# s12_j hosted between chunk k's c12 and f1 ops (j = k-1: mid-stream DVE
# is DMA-gated, so waiting on ACT's t1_{k-1} there is free)
INTER = {k: k - 1 for k in range(1, NCHUNK - 1)}
# s12s emitted after chunk k's f1 (the tail, where ACT lags)
POST = {NCHUNK - 1: [NCHUNK - 2, NCHUNK - 1]}
# sqrt grouping: (first_chunk, last_chunk) half-open chunk ranges, one
# accumulator column each; arranged so only a narrow sqrt lands in the tail
SQRT_GROUPS = [(0, 4), (4, 6), (6, 8)]
# sqrt group g is emitted after squares of chunk GRP_AFTER[g] (None =
# after all squares); chosen so wide sqrts don't gate on late chunks
GRP_AFTER = {0: 5, 1: 5, 2: None}
NCONST = 8                 # per-sample scalar constants

_CACHED_NC = None


def _quat_to_rot(q):
    """Normalized quaternion [w,x,y,z] -> 3x3 rotation matrix (float64)."""
    q = q / np.linalg.norm(q)
    w, x, y, z = q
    return np.array([
        [1 - 2*y*y - 2*z*z, 2*x*y - 2*z*w,     2*x*z + 2*y*w],
        [2*x*y + 2*z*w,     1 - 2*x*x - 2*z*z, 2*y*z - 2*x*w],
        [2*x*z - 2*y*w,     2*y*z + 2*x*w,     1 - 2*x*x - 2*y*y],
    ])


def _pivoted_qr(A3):
    """Column-pivoted QR of a 3x3 matrix (float64). A3[:, piv] = Q @ R.

    Modified Gram-Schmidt with greedy max-residual-norm pivoting, which
    guarantees |R[i, j]| <= |R[i, i]| for j > i (bounded ratios)."""
    cols = {c: A3[:, c].astype(np.float64).copy() for c in range(3)}
    coeff = {c: np.zeros(3) for c in range(3)}   # coeff[c][i] = Q[:,i].A3[:,c]
    remaining = [0, 1, 2]
    piv = []
    Q = np.zeros((3, 3))
    for i in range(3):
        cbest = max(remaining, key=lambda c: float(np.dot(cols[c], cols[c])))
        remaining.remove(cbest)
        piv.append(cbest)
        v = cols[cbest]
        nrm = np.sqrt(np.dot(v, v))
        if nrm < 1e-300:
            # Degenerate column: pick any unit vector orthogonal to prior qs.
            for basis in np.eye(3):
                w = basis - Q[:, :i] @ (Q[:, :i].T @ basis)
                if np.dot(w, w) > 1e-12:
                    v = w
                    break
            nrm = np.sqrt(np.dot(v, v))
        q = v / nrm
        Q[:, i] = q
        for c in [cbest] + remaining:
            proj = float(np.dot(q, cols[c]))
            coeff[c][i] = proj
            cols[c] = cols[c] - proj * q
    R = np.stack([coeff[c] for c in piv], axis=1)
    return Q, R, piv


def _per_sample_host(tt, tr, te, re_):
    """Returns (piv, consts[8] float32) for one sample."""
    R_t = _quat_to_rot(tr.astype(np.float64))
    R_p = _quat_to_rot(re_.astype(np.float64))
    A3 = R_p.T @ R_t - np.eye(3)
    a4 = R_p.T @ (tt.astype(np.float64) - te.astype(np.float64))
    Q, R, piv = _pivoted_qr(A3)
    b4 = Q.T @ a4
    r11, r12, r13 = R[0, 0], R[0, 1], R[0, 2]
    r22, r23 = R[1, 1], R[1, 2]
    alpha = r12 / r11 if abs(r11) > 1e-30 else 0.0
    beta = r13 / r11 if abs(r11) > 1e-30 else 0.0
    gamma = r23 / r22 if abs(r22) > 1e-30 else 0.0
    consts = np.array([alpha, beta, gamma, r11, b4[0], r22, b4[1],
                       b4[2] ** 2])
    return piv, consts


def _build_nc():
    """Raw-Bass kernel (no TileContext): all synchronization is standalone
    wait_ge instructions on explicit semaphores.

    Engine programs:
      Pool (SWDGE): consts DMA, then one point-cloud DMA per free-dim chunk
          (fp32->fp16 cast in flight), each signalling its own semaphore.
          SWDGE emits descriptors in FIFO order -> chunks land in order.
      DVE: per chunk, 3 tensor_scalar multiplies + 3 tensor_tensor adds
          (combines; f1 signals), plus the s12 = t1 + t2 add (signals).
          s12 of chunk k is emitted after the combines of chunk k+1 so DVE
          never stalls on ACT, and every same-engine RAW pair has >=1
          instruction in between (no drain needed).
      ACT: per chunk, 2 Square activations (fused per-partition scale+bias;
          t2 signals) and 1 Sqrt with bias b3^2 and accum_out -> acc column
          (signals).
      SP: waits for all Sqrt results, DMAs acc out, waits for completion.
    """
    import concourse.bass as bass
    from concourse import mybir
    from contextlib import ExitStack

    f16, f32 = mybir.dt.float16, mybir.dt.float32
    Alu = mybir.AluOpType
    Act = mybir.ActivationFunctionType

    nc = bass.Bass("TRN2", target_bir_lowering=False, debug=False,
                   num_devices=NCORES)
    # piece-major flat layout: piece j of chunk k stores 3 contiguous
    # [128, Pf] blocks (one per coord), so every dma_start reads one
    # sequential HBM range.
    pc = nc.dram_tensor("pc", [NPART * 3 * PPTS], f32,
                        kind="ExternalInput").ap()
    # per-partition scalars [128, 8]: (alpha, beta, gamma, s1, b1, s2, b2,
    # b3sq), fp32; constant within each sample's 32 partitions.
    consts = nc.dram_tensor("consts", [NPART, NCONST], f32,
                            kind="ExternalInput").ap()
    ngroups = len(SQRT_GROUPS)
    acc_out0 = nc.dram_tensor("acc0", [NPART, ngroups - 1], f32,
                              kind="ExternalOutput").ap()
    acc_out1 = nc.dram_tensor("acc1", [NPART, 1], f32,
                              kind="ExternalOutput").ap()

    with ExitStack() as ctx:
        E = ctx.enter_context
        ct = E(nc.sbuf_tensor("ct", [NPART, NCONST], f32))
        acc = E(nc.sbuf_tensor("acc_sb", [NPART, ngroups], f32))
        scr = E(nc.sbuf_tensor("scr", [NPART, 4], f16))

        def tiles(nm, mult=1):
            return [E(nc.sbuf_tensor(f"{nm}{k}", [NPART, mult * CHUNKS[k]],
                                     f16)) for k in range(NCHUNK)]

        us = tiles("u", 3)
        a1s = tiles("a1_")
        aa = tiles("aa_", 2)       # [a2 | a3]
        cc12 = tiles("c12_", 2)    # [c1 | c2]
        f1s = tiles("f1_")
        t1s = tiles("t1_")
        t2s = tiles("t2_")
        # s12 results for all chunks live in ONE tile so sqrt can span
        # several chunks in a single activation
        s12all = E(nc.sbuf_tensor("s12all", [NPART, PPTS], f16))
        esall = E(nc.sbuf_tensor("esall", [NPART, PPTS], f16))

        sem_u = [E(nc.semaphore(f"sem_u{k}")) for k in range(NCHUNK)]
        sem_dve = E(nc.semaphore("sem_dve"))
        sem_act = E(nc.semaphore("sem_act"))
        sem_out = E(nc.semaphore("sem_out"))
        block = E(nc.Block())

        def cst(i):
            return ct[:, i:i + 1]

        def s12sl(k):
            return s12all[:, OFFS[k]:OFFS[k + 1]]

        # --- engine emit orders (see SPLIT/INTER/POST/GRP_AFTER) ---
        act_order = []
        for k in range(NCHUNK):
            act_order.append(("sq", k))
            for gi in range(ngroups):
                if GRP_AFTER[gi] == k:
                    act_order.append(("grp", gi))
        for gi in range(ngroups):
            if GRP_AFTER[gi] is None:
                act_order.append(("grp", gi))

        # --- semaphore tick bookkeeping (program order per engine) ---
        dve_c2, dve_f1, dve_s12 = {}, {}, {}
        act_t1, act_grp = {}, {}
        dve_n = act_n = 0
        for k in range(NCHUNK):
            dve_n += 1; dve_c2[k] = dve_n
            if k in INTER:
                dve_n += 1; dve_s12[INTER[k]] = dve_n
            dve_n += 1; dve_f1[k] = dve_n
            for j in POST.get(k, []):
                dve_n += 1; dve_s12[j] = dve_n
        for op, idx in act_order:
            act_n += 1
            if op == "sq":
                act_t1[idx] = act_n
            else:
                act_grp[idx] = act_n

        @block.gpsimd
        def _(g):
            g.dma_start(ct[:], consts).then_inc(sem_u[0], 16)
            for k in range(NCHUNK):
                base = NPART * 3 * OFFS[k]
                g.dma_start(
                    us[k][:].rearrange("p (c f) -> p c f", c=3),
                    pc[base:base + 3 * NPART * CHUNKS[k]].rearrange(
                        "(c p f) -> p c f", c=3, p=NPART),
                ).then_inc(sem_u[k], 16)

        @block.vector
        def _(v):
            def s12(j):
                v.wait_ge(sem_act, act_t1[j])
                v.tensor_tensor(s12sl(j), t1s[j][:], t2s[j][:],
                                Alu.add).then_inc(sem_dve, 1)

            for k in range(NCHUNK):
                F = CHUNKS[k]
                u = us[k]
                u1, u2, u3 = (u[:, i * F:(i + 1) * F] for i in range(3))
                a2 = aa[k][:, 0:F]
                a3 = aa[k][:, F:2 * F]
                c1 = cc12[k][:, 0:F]
                c2 = cc12[k][:, F:2 * F]
                v.wait_ge(sem_u[k], 32 if k == 0 else 16)
                if k in SPLIT:
                    # split form; every same-engine RAW pair >=1 apart
                    v.tensor_scalar(a3, u3, cst(2), None, Alu.mult)
                    v.tensor_scalar(a2, u3, cst(1), None, Alu.mult)
                    v.tensor_scalar(a1s[k][:], u2, cst(0), None, Alu.mult)
                    v.tensor_tensor(c1, u1, a2, Alu.add)
                    v.tensor_tensor(c2, u2, a3, Alu.add).then_inc(sem_dve, 1)
                    v.tensor_tensor(f1s[k][:], c1, a1s[k][:],
                                    Alu.add).then_inc(sem_dve, 1)
                else:
                    # merged form: one add produces [c1|c2]; the
                    # interleaved s12 spaces the c12->f1 RAW pair
                    v.tensor_scalar(a2, u3, cst(1), None, Alu.mult)
                    v.tensor_scalar(a3, u3, cst(2), None, Alu.mult)
                    v.tensor_scalar(a1s[k][:], u2, cst(0), None, Alu.mult)
                    v.tensor_tensor(cc12[k][:], u[:, 0:2 * F], aa[k][:],
                                    Alu.add).then_inc(sem_dve, 1)
                    s12(INTER[k])
                    v.tensor_tensor(f1s[k][:], c1, a1s[k][:],
                                    Alu.add).then_inc(sem_dve, 1)
                for j in POST.get(k, []):
                    s12(j)

        @block.scalar
        def _(s):
            # dummy activations preload the Square/Sqrt tables while the
            # first DMA is still in flight
            s.activation(scr[:, 2:4], scr[:, 0:2], Act.Square)
            s.activation(scr[:, 0:2], scr[:, 0:2], Act.Sqrt)

            def squares(k):
                F = CHUNKS[k]
                s.wait_ge(sem_dve, dve_c2[k])
                s.activation(t2s[k][:], cc12[k][:, F:2 * F], Act.Square,
                             bias=cst(6), scale=cst(5))
                s.wait_ge(sem_dve, dve_f1[k])
                s.activation(t1s[k][:], f1s[k][:], Act.Square,
                             bias=cst(4), scale=cst(3)).then_inc(sem_act, 1)

            def sqrt_grp(gi):
                a, bb = SQRT_GROUPS[gi]
                s.wait_ge(sem_dve, max(dve_s12[k] for k in range(a, bb)))
                s.activation(esall[:, OFFS[a]:OFFS[bb]],
                             s12all[:, OFFS[a]:OFFS[bb]], Act.Sqrt,
                             bias=cst(7),
                             accum_out=acc[:, gi:gi + 1]).then_inc(sem_act, 1)

            for op, idx in act_order:
                if op == "sq":
                    squares(idx)
                else:
                    sqrt_grp(idx)

        @block.sync
        def _(sp):
            # first accumulator columns go out as soon as their sqrt
            # groups are done; the last column right at the end
            half = ngroups - 1
            sp.wait_ge(sem_act, act_grp[half - 1])
            sp.dma_start(acc_out0, acc[:, 0:half]).then_inc(sem_out, 16)
            sp.wait_ge(sem_act, act_grp[ngroups - 1])
            sp.dma_start(acc_out1, acc[:, half:]).then_inc(sem_out, 16)
            sp.wait_ge(sem_out, 32)

    return nc


def _get_nc():
    global _CACHED_NC
    if _CACHED_NC is None:
        _CACHED_NC = _build_nc()
    return _CACHED_NC


def _kernel_impl(point_clouds, target_transl, target_rot, transl_err, rot_err,
                 trace=False):
    from concourse.bass_utils import run_bass_kernel_spmd

    pc = np.asarray(point_clouds)
    tt = np.asarray(target_transl, np.float64)
    tr = np.asarray(target_rot, np.float64)
    te = np.asarray(transl_err, np.float64)
    re_ = np.asarray(rot_err, np.float64)

    # ---- pose loss (host, float64, exact reference formulas) ----
    d = np.abs(te - tt)
    loss_transl = np.where(d < 1.0, 0.5 * d * d, d - 0.5).sum(axis=1).mean()

    rinv = tr * np.array([1.0, -1.0, -1.0, -1.0])
    q = re_
    w = q[:, 0]*rinv[:, 0] - q[:, 1]*rinv[:, 1] - q[:, 2]*rinv[:, 2] - q[:, 3]*rinv[:, 3]
    x = q[:, 0]*rinv[:, 1] + q[:, 1]*rinv[:, 0] + q[:, 2]*rinv[:, 3] - q[:, 3]*rinv[:, 2]
    y = q[:, 0]*rinv[:, 2] - q[:, 1]*rinv[:, 3] + q[:, 2]*rinv[:, 0] + q[:, 3]*rinv[:, 1]
    z = q[:, 0]*rinv[:, 3] + q[:, 1]*rinv[:, 2] - q[:, 2]*rinv[:, 1] + q[:, 3]*rinv[:, 0]
    angle = 2.0 * np.arctan2(np.sqrt(x*x + y*y + z*z), np.abs(w))
    loss_rot = (180.0 * angle / np.pi).mean()
    pose_loss = loss_transl + loss_rot

    # ---- per-sample transform constants (host) ----
    all_consts = np.zeros((B, NCONST), np.float32)
    all_piv = []
    for b in range(B):
        piv, consts = _per_sample_host(tt[b], tr[b], te[b], re_[b])
        all_consts[b] = consts
        all_piv.append(piv)

    # ---- build per-core inputs (permute coord rows per pivoting, pack
    #      4 samples x 32 partition-rows x 6250 points, chunk-major) ----
    pcp = np.stack([pc[b, all_piv[b], :] for b in range(B)])   # [B,3,N]
    pcp = pcp.reshape(NCORES, SPC, 3, ROWS, PPTS).transpose(0, 1, 3, 2, 4)
    pcp = pcp.reshape(NCORES, NPART, 3, PPTS)
    # chunk-major flat: [concat over chunks of [3, 128, F]]
    pcf = np.concatenate(
        [pcp[:, :, :, OFFS[k]:OFFS[k + 1]].transpose(0, 2, 1, 3)
         .reshape(NCORES, -1) for k in range(NCHUNK)], axis=1)
    in_maps = []
    for k in range(NCORES):
        cc = all_consts[k * SPC:(k + 1) * SPC]                 # [SPC, 8]
        in_maps.append({
            "pc": np.ascontiguousarray(pcf[k]),
            "consts": np.repeat(cc, ROWS, axis=0),             # [128, 8]
        })

    nc = _get_nc()
    res = run_bass_kernel_spmd(nc, in_maps, core_ids=list(range(NCORES)),
                               trace=trace)

    # ---- combine (host, float64) ----
    pcl_sum = 0.0
    for k in range(NCORES):
        pcl_sum += (res.results[k]["acc0"].astype(np.float64).sum()
                    + res.results[k]["acc1"].astype(np.float64).sum()) / N

    total = 0.5 * pose_loss + 0.5 * (pcl_sum / B)
    out = (np.float32(total), np.float32(loss_transl), np.float32(loss_rot),
           np.float32(pcl_sum / B))
    return out, res


def kernel(point_clouds, target_transl, target_rot, transl_err, rot_err):
    out, _ = _kernel_impl(point_clouds, target_transl, target_rot,
                          transl_err, rot_err)
    return out


# revision 35
# speedup vs baseline: 1.3673x; 1.3331x over previous
"""Trainium2 Bass kernel for nn_CombinedLoss (LCCNet CombinedLoss).

Strategy
--------
The only heavy part is the point-cloud term: for each sample b,
    err_n = || (RT_inv_b - I) @ p_n ||   over N=200000 homogeneous points,
    loss_pc = sum_b mean_n err_n.
Everything else (pose loss, 4x4 transform algebra) is O(B) scalar work done
on the host in float64.

Per sample the displacement is d = A3 @ q + a4 with q = (x,y,z). Using a
column-pivoted QR A3 = Q R (orthogonal Q preserves the norm):
    err^2 = (s1*(u1 + a*u2 + b*u3) + b1)^2
          + (s2*(u2 + g*u3) + b2)^2
          + b3^2                      (A3 is rank 2 -> R[2,2] = 0)
where u = permuted coords, s_i = R[i,i], (b1,b2,b3) = Q^T a4, and the ratios
a,b,g are bounded by 1 thanks to pivoting (fp16-safe).

Device mapping (8 cores, data-parallel over batch; per core 4 samples packed
as 128 partitions = 4 samples x 32 partition-rows, 6250 points per row):
  - The per-sample scalars live in a [128, 1] column each (constant within a
    sample's 32 partitions), so ONE instruction covers all 4 samples.
  - DMA (gpsimd SWDGE, fp32->fp16 cast in flight) streams the free dim in
    chunks; compute starts when chunk 0 lands.
  - DVE per chunk: 3 tensor_scalar multiplies (4x perf mode) + 3
    tensor_tensor adds (2x mode) for the combines, + 1 add for s12.
    Instruction order keeps >=1 op between same-engine RAW pairs, so no
    drain() is needed.
  - ACT per chunk: 2 Square activations (fused per-partition scale+bias) +
    1 Sqrt with per-partition bias b3^2 and free-dim accum_out.
  - SP: waits for all Sqrts, DMAs the [128, n_chunks] accumulator out.
  - Host: final sums in float64, pose loss, combine.
"""

import numpy as np

B = 32
N = 200000
NCORES = 8
SPC = B // NCORES          # samples per core
NPART = 128
ROWS = 32                  # partition-rows per sample
PPTS = N // ROWS           # points per partition-row = 6250
# DMA chunks (fp16, host-cast): the host packs per-core fp16 buffers, so
# the stream is 4.8MB/core instead of 9.6MB and runs on the fast HWDGE
# (SP) ring with no in-flight cast. One dma_start per DMA chunk.
DMAC = [768, 2504, 2504, 474]
# compute chunks subdivide DMA chunks (no extra semaphores needed)
CHUNKS = [768, 1252, 1252, 1252, 1252, 474]
DMACHUNK = [0, 1, 1, 2, 2, 3]      # compute chunk -> DMA chunk
NCHUNK = len(CHUNKS)
OFFS = [0]
for _f in CHUNKS:
    OFFS.append(OFFS[-1] + _f)
DOFFS = [0]
for _f in DMAC:
    DOFFS.append(DOFFS[-1] + _f)
# chunks using the 7-op split form (separate c1/c2 adds); the others use
# the 6-op merged form. Both may host an interleaved s12 between c2/c12
# and f1 (which also spaces the same-engine RAW pair).
SPLIT = {0, NCHUNK - 1}
# s12_j hosted inside chunk k's group (between c12 and f1): ~1.5 chunks
# of slack so neither DVE nor the sqrt groups wait long
INTER = {2: 0, 3: 1, 4: 2, 5: 3}
# s12s emitted after chunk k's f1 (the tail)
POST = {NCHUNK - 1: [4, 5]}
# sqrt grouping: (first_chunk, last_chunk) half-open chunk ranges, one
# accumulator column each
SQRT_GROUPS = [(0, 2), (2, 4), (4, 6)]
# sqrt group g is emitted after squares of chunk GRP_AFTER[g] (None =
# after all squares)
GRP_AFTER = {0: 3, 1: 4, 2: None}
NCONST = 8                 # per-sample scalar constants

_CACHED_NC = None


def _quat_to_rot(q):
    """Normalized quaternion [w,x,y,z] -> 3x3 rotation matrix (float64)."""
    q = q / np.linalg.norm(q)
    w, x, y, z = q
    return np.array([
        [1 - 2*y*y - 2*z*z, 2*x*y - 2*z*w,     2*x*z + 2*y*w],
        [2*x*y + 2*z*w,     1 - 2*x*x - 2*z*z, 2*y*z - 2*x*w],
        [2*x*z - 2*y*w,     2*y*z + 2*x*w,     1 - 2*x*x - 2*y*y],
    ])


def _pivoted_qr(A3):
    """Column-pivoted QR of a 3x3 matrix (float64). A3[:, piv] = Q @ R.

    Modified Gram-Schmidt with greedy max-residual-norm pivoting, which
    guarantees |R[i, j]| <= |R[i, i]| for j > i (bounded ratios)."""
    cols = {c: A3[:, c].astype(np.float64).copy() for c in range(3)}
    coeff = {c: np.zeros(3) for c in range(3)}   # coeff[c][i] = Q[:,i].A3[:,c]
    remaining = [0, 1, 2]
    piv = []
    Q = np.zeros((3, 3))
    for i in range(3):
        cbest = max(remaining, key=lambda c: float(np.dot(cols[c], cols[c])))
        remaining.remove(cbest)
        piv.append(cbest)
        v = cols[cbest]
        nrm = np.sqrt(np.dot(v, v))
        if nrm < 1e-300:
            # Degenerate column: pick any unit vector orthogonal to prior qs.
            for basis in np.eye(3):
                w = basis - Q[:, :i] @ (Q[:, :i].T @ basis)
                if np.dot(w, w) > 1e-12:
                    v = w
                    break
            nrm = np.sqrt(np.dot(v, v))
        q = v / nrm
        Q[:, i] = q
        for c in [cbest] + remaining:
            proj = float(np.dot(q, cols[c]))
            coeff[c][i] = proj
            cols[c] = cols[c] - proj * q
    R = np.stack([coeff[c] for c in piv], axis=1)
    return Q, R, piv


def _per_sample_host(tt, tr, te, re_):
    """Returns (piv, consts[8] float32) for one sample."""
    R_t = _quat_to_rot(tr.astype(np.float64))
    R_p = _quat_to_rot(re_.astype(np.float64))
    A3 = R_p.T @ R_t - np.eye(3)
    a4 = R_p.T @ (tt.astype(np.float64) - te.astype(np.float64))
    Q, R, piv = _pivoted_qr(A3)
    b4 = Q.T @ a4
    r11, r12, r13 = R[0, 0], R[0, 1], R[0, 2]
    r22, r23 = R[1, 1], R[1, 2]
    alpha = r12 / r11 if abs(r11) > 1e-30 else 0.0
    beta = r13 / r11 if abs(r11) > 1e-30 else 0.0
    gamma = r23 / r22 if abs(r22) > 1e-30 else 0.0
    consts = np.array([alpha, beta, gamma, r11, b4[0], r22, b4[1],
                       b4[2] ** 2])
    return piv, consts


def _build_nc():
    """Raw-Bass kernel (no TileContext): all synchronization is standalone
    wait_ge instructions on explicit semaphores.

    Engine programs:
      Pool (SWDGE): consts DMA, then one point-cloud DMA per free-dim chunk
          (fp32->fp16 cast in flight), each signalling its own semaphore.
          SWDGE emits descriptors in FIFO order -> chunks land in order.
      DVE: per chunk, 3 tensor_scalar multiplies + 3 tensor_tensor adds
          (combines; f1 signals), plus the s12 = t1 + t2 add (signals).
          s12 of chunk k is emitted after the combines of chunk k+1 so DVE
          never stalls on ACT, and every same-engine RAW pair has >=1
          instruction in between (no drain needed).
      ACT: per chunk, 2 Square activations (fused per-partition scale+bias;
          t2 signals) and 1 Sqrt with bias b3^2 and accum_out -> acc column
          (signals).
      SP: waits for all Sqrt results, DMAs acc out, waits for completion.
    """
    import concourse.bass as bass
    from concourse import mybir
    from contextlib import ExitStack

    f16, f32 = mybir.dt.float16, mybir.dt.float32
    Alu = mybir.AluOpType
    Act = mybir.ActivationFunctionType

    nc = bass.Bass("TRN2", target_bir_lowering=False, debug=False,
                   num_devices=NCORES)
    # fp16 host-cast points, chunk-major flat: DMA chunk d is a contiguous
    # [3, 128, DMAC[d]] fp16 block (coord, partition, point), so each
    # dma_start reads one sequential HBM range. No in-flight cast -> the
    # fast HWDGE (SP) ring issues everything.
    pc = nc.dram_tensor("pc", [NPART * 3 * PPTS], f16,
                        kind="ExternalInput").ap()
    # per-partition scalars [128, 8]: (alpha, beta, gamma, s1, b1, s2, b2,
    # b3sq), fp32; constant within each sample's 32 partitions.
    consts = nc.dram_tensor("consts", [NPART, NCONST], f32,
                            kind="ExternalInput").ap()
    ngroups = len(SQRT_GROUPS)
    acc_out = nc.dram_tensor("acc", [NPART, ngroups], f32,
                             kind="ExternalOutput").ap()

    with ExitStack() as ctx:
        E = ctx.enter_context
        ct = E(nc.sbuf_tensor("ct", [NPART, NCONST], f32))
        acc = E(nc.sbuf_tensor("acc_sb", [NPART, ngroups], f32))
        scr = E(nc.sbuf_tensor("scr", [NPART, 4], f16))

        # point tiles are allocated per DMA chunk, [u1 | u2 | u3] with the
        # full DMA-chunk width per coord; compute chunks slice into them
        ud = [E(nc.sbuf_tensor(f"u{d}", [NPART, 3 * DMAC[d]], f16))
              for d in range(len(DMAC))]

        def tiles(nm, mult=1):
            return [E(nc.sbuf_tensor(f"{nm}{k}", [NPART, mult * CHUNKS[k]],
                                     f16)) for k in range(NCHUNK)]

        a1s = tiles("a1_")
        aa = tiles("aa_", 2)       # [a2 | a3]
        cc12 = tiles("c12_", 2)    # [c1 | c2]
        f1s = tiles("f1_")
        t1s = tiles("t1_")
        t2s = tiles("t2_")
        # s12 results for all chunks live in ONE tile so sqrt can span
        # several chunks in a single activation
        s12all = E(nc.sbuf_tensor("s12all", [NPART, PPTS], f16))
        esall = E(nc.sbuf_tensor("esall", [NPART, PPTS], f16))

        sem_u = [E(nc.semaphore(f"sem_u{d}")) for d in range(len(DMAC))]
        sem_dve = E(nc.semaphore("sem_dve"))
        sem_act = E(nc.semaphore("sem_act"))
        sem_out = E(nc.semaphore("sem_out"))
        block = E(nc.Block())

        def cst(i):
            return ct[:, i:i + 1]

        def s12sl(k):
            return s12all[:, OFFS[k]:OFFS[k + 1]]

        def usl(k, i):
            """coord-i slice of compute chunk k inside its DMA-chunk tile"""
            d = DMACHUNK[k]
            lo = OFFS[k] - DOFFS[d]
            return ud[d][:, i * DMAC[d] + lo:i * DMAC[d] + lo + CHUNKS[k]]

        # --- engine emit orders (see SPLIT/INTER/POST/GRP_AFTER) ---
        act_order = []
        for k in range(NCHUNK):
            act_order.append(("sq", k))
            for gi in range(ngroups):
                if GRP_AFTER[gi] == k:
                    act_order.append(("grp", gi))
        for gi in range(ngroups):
            if GRP_AFTER[gi] is None:
                act_order.append(("grp", gi))

        # --- semaphore tick bookkeeping (program order per engine) ---
        dve_c2, dve_f1, dve_s12 = {}, {}, {}
        act_t1, act_grp = {}, {}
        dve_n = act_n = 0
        for k in range(NCHUNK):
            dve_n += 1; dve_c2[k] = dve_n
            if k in INTER:
                dve_n += 1; dve_s12[INTER[k]] = dve_n
            dve_n += 1; dve_f1[k] = dve_n
            for j in POST.get(k, []):
                dve_n += 1; dve_s12[j] = dve_n
        for op, idx in act_order:
            act_n += 1
            if op == "sq":
                act_t1[idx] = act_n
            else:
                act_grp[idx] = act_n

        @block.vector
        def _(v):
            def s12(j):
                v.wait_ge(sem_act, act_t1[j])
                v.tensor_tensor(s12sl(j), t1s[j][:], t2s[j][:],
                                Alu.add).then_inc(sem_dve, 1)

            last_d = -1
            for k in range(NCHUNK):
                F = CHUNKS[k]
                u1, u2, u3 = usl(k, 0), usl(k, 1), usl(k, 2)
                a2 = aa[k][:, 0:F]
                a3 = aa[k][:, F:2 * F]
                c1 = cc12[k][:, 0:F]
                c2 = cc12[k][:, F:2 * F]
                d = DMACHUNK[k]
                if d != last_d:
                    # consts DMA also incs sem_u[0], hence the +16
                    v.wait_ge(sem_u[d], 32 if d == 0 else 16)
                    last_d = d
                if k in SPLIT:
                    # split form; every same-engine RAW pair >=1 apart
                    v.tensor_scalar(a3, u3, cst(2), None, Alu.mult)
                    v.tensor_scalar(a2, u3, cst(1), None, Alu.mult)
                    v.tensor_scalar(a1s[k][:], u2, cst(0), None, Alu.mult)
                    v.tensor_tensor(c1, u1, a2, Alu.add)
                    v.tensor_tensor(c2, u2, a3, Alu.add).then_inc(sem_dve, 1)
                    if k in INTER:
                        s12(INTER[k])
                    v.tensor_tensor(f1s[k][:], c1, a1s[k][:],
                                    Alu.add).then_inc(sem_dve, 1)
                else:
                    # merged form: one add produces [c1|c2]; u1,u2 are
                    # adjacent inside the DMA-chunk tile only when the
                    # compute chunk spans the whole DMA chunk, so build
                    # the pair via a 3D AP over (coord, point) instead
                    v.tensor_scalar(a2, u3, cst(1), None, Alu.mult)
                    v.tensor_scalar(a3, u3, cst(2), None, Alu.mult)
                    v.tensor_scalar(a1s[k][:], u2, cst(0), None, Alu.mult)
                    v.tensor_tensor(c1, u1, a2, Alu.add)
                    v.tensor_tensor(c2, u2, a3, Alu.add).then_inc(sem_dve, 1)
                    if k in INTER:
                        s12(INTER[k])
                    v.tensor_tensor(f1s[k][:], c1, a1s[k][:],
                                    Alu.add).then_inc(sem_dve, 1)
                for j in POST.get(k, []):
                    s12(j)

        @block.scalar
        def _(s):
            # dummy activations preload the Square/Sqrt tables while the
            # first DMA is still in flight
            s.activation(scr[:, 2:4], scr[:, 0:2], Act.Square)
            s.activation(scr[:, 0:2], scr[:, 0:2], Act.Sqrt)

            def squares(k):
                F = CHUNKS[k]
                s.wait_ge(sem_dve, dve_c2[k])
                s.activation(t2s[k][:], cc12[k][:, F:2 * F], Act.Square,
                             bias=cst(6), scale=cst(5))
                s.wait_ge(sem_dve, dve_f1[k])
                s.activation(t1s[k][:], f1s[k][:], Act.Square,
                             bias=cst(4), scale=cst(3)).then_inc(sem_act, 1)

            def sqrt_grp(gi):
                a, bb = SQRT_GROUPS[gi]
                s.wait_ge(sem_dve, max(dve_s12[k] for k in range(a, bb)))
                s.activation(esall[:, OFFS[a]:OFFS[bb]],
                             s12all[:, OFFS[a]:OFFS[bb]], Act.Sqrt,
                             bias=cst(7),
                             accum_out=acc[:, gi:gi + 1]).then_inc(sem_act, 1)

            for op, idx in act_order:
                if op == "sq":
                    squares(idx)
                else:
                    sqrt_grp(idx)

        @block.sync
        def _(sp):
            sp.dma_start(ct[:], consts).then_inc(sem_u[0], 16)
            for d in range(len(DMAC)):
                base = NPART * 3 * DOFFS[d]
                sp.dma_start(
                    ud[d][:].rearrange("p (c f) -> p c f", c=3),
                    pc[base:base + 3 * NPART * DMAC[d]].rearrange(
                        "(c p f) -> p c f", c=3, p=NPART),
                ).then_inc(sem_u[d], 16)
            sp.wait_ge(sem_act, act_n)
            sp.dma_start(acc_out, acc[:]).then_inc(sem_out, 16)
            sp.wait_ge(sem_out, 16)

    return nc


def _get_nc():
    global _CACHED_NC
    if _CACHED_NC is None:
        _CACHED_NC = _build_nc()
    return _CACHED_NC


def _kernel_impl(point_clouds, target_transl, target_rot, transl_err, rot_err,
                 trace=False):
    from concourse.bass_utils import run_bass_kernel_spmd

    pc = np.asarray(point_clouds)
    tt = np.asarray(target_transl, np.float64)
    tr = np.asarray(target_rot, np.float64)
    te = np.asarray(transl_err, np.float64)
    re_ = np.asarray(rot_err, np.float64)

    # ---- pose loss (host, float64, exact reference formulas) ----
    d = np.abs(te - tt)
    loss_transl = np.where(d < 1.0, 0.5 * d * d, d - 0.5).sum(axis=1).mean()

    rinv = tr * np.array([1.0, -1.0, -1.0, -1.0])
    q = re_
    w = q[:, 0]*rinv[:, 0] - q[:, 1]*rinv[:, 1] - q[:, 2]*rinv[:, 2] - q[:, 3]*rinv[:, 3]
    x = q[:, 0]*rinv[:, 1] + q[:, 1]*rinv[:, 0] + q[:, 2]*rinv[:, 3] - q[:, 3]*rinv[:, 2]
    y = q[:, 0]*rinv[:, 2] - q[:, 1]*rinv[:, 3] + q[:, 2]*rinv[:, 0] + q[:, 3]*rinv[:, 1]
    z = q[:, 0]*rinv[:, 3] + q[:, 1]*rinv[:, 2] - q[:, 2]*rinv[:, 1] + q[:, 3]*rinv[:, 0]
    angle = 2.0 * np.arctan2(np.sqrt(x*x + y*y + z*z), np.abs(w))
    loss_rot = (180.0 * angle / np.pi).mean()
    pose_loss = loss_transl + loss_rot

    # ---- per-sample transform constants (host) ----
    all_consts = np.zeros((B, NCONST), np.float32)
    all_piv = []
    for b in range(B):
        piv, consts = _per_sample_host(tt[b], tr[b], te[b], re_[b])
        all_consts[b] = consts
        all_piv.append(piv)

    # ---- build per-core inputs (permute coord rows per pivoting, pack
    #      4 samples x 32 partition-rows x 6250 points, chunk-major) ----
    pcp = np.stack([pc[b, all_piv[b], :] for b in range(B)])   # [B,3,N]
    pcp = pcp.reshape(NCORES, SPC, 3, ROWS, PPTS).transpose(0, 1, 3, 2, 4)
    pcp = pcp.reshape(NCORES, NPART, 3, PPTS)
    # fp16 host cast + DMA-chunk-major flat: [concat over DMA chunks of
    # [3, 128, Fd]] (halves the HBM stream and lets HWDGE issue the DMAs)
    pcp16 = pcp.astype(np.float16)
    pcf = np.concatenate(
        [pcp16[:, :, :, DOFFS[d]:DOFFS[d + 1]].transpose(0, 2, 1, 3)
         .reshape(NCORES, -1) for d in range(len(DMAC))], axis=1)
    in_maps = []
    for k in range(NCORES):
        cc = all_consts[k * SPC:(k + 1) * SPC]                 # [SPC, 8]
        in_maps.append({
            "pc": np.ascontiguousarray(pcf[k]),
            "consts": np.repeat(cc, ROWS, axis=0),             # [128, 8]
        })

    nc = _get_nc()
    res = run_bass_kernel_spmd(nc, in_maps, core_ids=list(range(NCORES)),
                               trace=trace)

    # ---- combine (host, float64) ----
    pcl_sum = 0.0
    for k in range(NCORES):
        pcl_sum += res.results[k]["acc"].astype(np.float64).sum() / N

    total = 0.5 * pose_loss + 0.5 * (pcl_sum / B)
    out = (np.float32(total), np.float32(loss_transl), np.float32(loss_rot),
           np.float32(pcl_sum / B))
    return out, res


def kernel(point_clouds, target_transl, target_rot, transl_err, rot_err):
    out, _ = _kernel_impl(point_clouds, target_transl, target_rot,
                          transl_err, rot_err)
    return out
